# revision 20
# baseline (speedup 1.0000x reference)
"""Trainium2 Bass kernel for nn_CognitiveModule (gnn_message_passing).

Computes, for L=8 layers of a 1536x1536 grid:
  internal = conv2d(prev_spikes, local_kernel, SAME)      # 11x11 distance kernel
  axonal   = segment_sum(prev_spikes[conn_src] * inter_weights, conn_dst)
  total    = external + internal + axonal
  active   = (refractory == 0)
  v_new    = 0.9 * membrane + active * total
  spikes   = (v_new > 0) * active          (the sigmoid straight-through term
                                            cancels in the forward pass)

Strategy (8 NeuronCores, shard H):
  - Each core gets 192 rows of every layer (plus a 5-row halo of prev_spikes).
  - Conv runs on the TensorEngine as banded matmuls over the row (partition)
    dimension: for each kernel column kx, a [106,96] band matrix contracts 106
    input rows into 96 output rows.  The x-taps are reduced from 11 to 6
    matmul passes via the kernel's x-symmetry: the DVE pre-adds the shifted
    spike images (S_d = X_{-d} + X_{+d}; spikes are {0,1} so sums are exact
    in bf16).  A one-column-shifted copy (Xo) keeps every DVE op 4B-aligned
    (2x perf mode).
  - All matmul data is bf16.  Kernel/weight values are split hi/lo
    (w = bf16(w) + bf16(w - bf16(w))) so products against {0,1,2} spikes are
    exact to ~2^-18 relative - fp32-class accuracy at bf16 matmul speed.
  - external + 0.9*membrane and the refractory gate are folded on the host
    into one fp32 threshold plane  thr = BIG*(refr != 0) - (ext + 0.9*mem);
    the device finalize is ONE VectorEngine op per row-block:
        out = (psum > thr)  as {1.0, 0.0}  (bf16, cast to fp32 on the host).
  - Axonal products are computed on the VectorEngine from the resident spike
    tiles (over all 106 rows) and accumulated into PSUM with a shifted
    identity matmul (iden5: rows 5..100 -> psum rows 0..95).  Connections
    with src >= dst get a dedicated spike load at the start of each
    row-block.  The finalize is software-pipelined one layer behind the
    matmuls so the PE never waits on the DVE queue.
"""

import sys

for _p in ("/opt/trn_rl_repo", "/root/.axon_site/_ro/trn_rl_repo"):
    if _p not in sys.path:
        sys.path.append(_p)

import numpy as np
import ml_dtypes

import concourse.bass as bass
import concourse.mybir as mybir
import concourse.tile as tile
from concourse import bacc
from concourse.bass_utils import run_bass_kernel_spmd

BF16 = mybir.dt.bfloat16
F32 = mybir.dt.float32
BIG = np.float32(1.0e5)
DECAY = np.float32(0.9)

L = 8
NCORES = 8
TH = 96          # output rows per conv tile
HALO = 5
KS = 11          # kernel size
KR = TH + 2 * HALO  # 106 input rows per conv tile
NFREE = 512      # psum free-dim tile
WPAD = 12        # spk row padding: 5 left + 7 right (even widths, alignment)


def _split_bf16(x):
    hi = x.astype(ml_dtypes.bfloat16)
    lo = (x - hi.astype(np.float32)).astype(ml_dtypes.bfloat16)
    return hi, lo


def _group_kernel_columns(kern):
    """Group the 11 kernel columns by x-symmetry: ("pair", d) groups read the
    pre-added S_d image; ("single", dx) groups read a shifted X window."""
    groups = []
    used = [False] * KS
    for d in range(0, HALO + 1):
        a, b = HALO + d, HALO - d
        if d == 0:
            groups.append(("pair", 0, kern[:, HALO].copy()))
            used[HALO] = True
        elif np.array_equal(kern[:, a], kern[:, b]):
            groups.append(("pair", d, kern[:, a].copy()))
            used[a] = used[b] = True
    for kx in range(KS):
        if not used[kx]:
            groups.append(("single", kx - HALO, kern[:, kx].copy()))
    return groups


def _band_matrix(col):
    """[KR, TH] band matrix: B[k, m] = col[k - m] for 0 <= k-m <= 10.
    X partition k holds spike row r0 + k - 5 (straight layout)."""
    B = np.zeros((KR, TH), np.float32)
    for m in range(TH):
        for ky in range(KS):
            B[m + ky, m] = col[ky]
    return B


def _build_program(conns, R, W, groups_meta):
    """Build the SPMD Bass program (identical on all cores)."""
    nc = bacc.Bacc(None, target_bir_lowering=False, debug=False)
    NT = W // NFREE
    HT = R // TH
    NG = len(groups_meta)

    spk_d = nc.dram_tensor("spk", [L, R + 2 * HALO, W + WPAD], BF16,
                           kind="ExternalInput")
    thr_d = nc.dram_tensor("thr", [L, R, W], F32, kind="ExternalInput")
    # w hi/lo packed side by side, rows padded by the conv halo:
    # [C, 5 + r + 5, 0:W] = hi(row r), [.., W:2W] = lo(row r)
    wpk_d = nc.dram_tensor("wpk", [len(conns), R + 2 * HALO, 2 * W], BF16,
                           kind="ExternalInput")
    bands_d = nc.dram_tensor("bands", [KR, 2 * NG * TH], BF16,
                             kind="ExternalInput")
    iden_d = nc.dram_tensor("iden", [KR, TH], BF16, kind="ExternalInput")
    out_d = nc.dram_tensor("out", [L, R, W], BF16, kind="ExternalOutput")

    pre_conns = [i for i, (s, d) in enumerate(conns) if s >= d]
    inline_conns = [i for i, (s, d) in enumerate(conns) if s < d]
    by_src = {}
    for i in inline_conns:
        by_src.setdefault(conns[i][0], []).append(i)
    by_dst = {}
    for i in range(len(conns)):
        by_dst.setdefault(conns[i][1], []).append(i)

    with tile.TileContext(nc) as tc:
        with (
            tc.tile_pool(name="const", bufs=1) as constp,
            tc.tile_pool(name="xp", bufs=3) as xp,
            tc.tile_pool(name="sp", bufs=2) as sp,
            tc.tile_pool(name="thrp", bufs=3) as thrp,
            tc.tile_pool(name="wp", bufs=6) as wp,
            tc.tile_pool(name="cp", bufs=12) as cp,
            tc.tile_pool(name="op", bufs=3) as op,
            tc.tile_pool(name="prep", bufs=2) as prep,
            tc.tile_pool(name="ps", bufs=2, space="PSUM") as psp,
        ):
            bands_sb = constp.tile([KR, 2 * NG * TH], BF16)
            nc.sync.dma_start(out=bands_sb[:], in_=bands_d[:])
            iden_sb = constp.tile([KR, TH], BF16)
            nc.sync.dma_start(out=iden_sb[:], in_=iden_d[:])

            # deferred finalize, one layer behind (PE never waits on DVE)
            pending = [None]

            def flush_pending():
                if pending[0] is None:
                    return
                ps_p, thr_p, out_p, l_p, r0_p = pending[0]
                nc.vector.tensor_tensor(
                    out=out_p[:], in0=ps_p[:], in1=thr_p[:],
                    op=mybir.AluOpType.is_gt)
                nc.sync.dma_start(out=out_d[l_p, r0_p:r0_p + TH, :],
                                  in_=out_p[:])
                pending[0] = None

            for h in range(HT):
                r0 = h * TH
                contrib = {}  # conn idx -> (hi_tile, lo_tile)

                # connections whose src comes later in the layer loop: load
                # the needed spike rows now (odd-aligned, all 106 rows).
                for ci in pre_conns:
                    s = conns[ci][0]
                    spre = prep.tile([KR, W + 8], BF16, tag="spre")
                    nc.scalar.dma_start(
                        out=spre[:],
                        in_=spk_d[s, r0:r0 + KR, 1:W + 9])
                    wt = wp.tile([KR, 2 * W], BF16, tag="w")
                    nc.scalar.dma_start(out=wt[:],
                                        in_=wpk_d[ci, r0:r0 + KR, :])
                    chi = cp.tile([KR, W], BF16, tag="c")
                    clo = cp.tile([KR, W], BF16, tag="c")
                    xs = spre[:, 4:4 + W]
                    nc.vector.tensor_tensor(out=chi[:], in0=xs,
                                            in1=wt[:, 0:W],
                                            op=mybir.AluOpType.mult)
                    nc.vector.tensor_tensor(out=clo[:], in0=xs,
                                            in1=wt[:, W:2 * W],
                                            op=mybir.AluOpType.mult)
                    contrib[ci] = (chi, clo)

                for l in range(L):
                    # X[k] = spike row r0 + k - 5 (dram row r0 + k), cols at
                    # byte 2x.  Xo is shifted one column (odd alignment) for
                    # the even-d pre-adds and the contrib muls.
                    X = xp.tile([KR, W + 2 * HALO], BF16, tag="X")
                    nc.scalar.dma_start(
                        out=X[:], in_=spk_d[l, r0:r0 + KR, 0:W + 2 * HALO])
                    Xo = xp.tile([KR, W + 8], BF16, tag="Xo")
                    nc.scalar.dma_start(
                        out=Xo[:], in_=spk_d[l, r0:r0 + KR, 1:W + 9])

                    # symmetric pre-adds S_d = X_{-d} + X_{+d} (all DVE 2x)
                    svec = {}
                    for gi, (kind, d) in enumerate(groups_meta):
                        if kind == "pair" and d > 0:
                            S = sp.tile([KR, W], BF16, tag=f"S{d}")
                            if d % 2 == 0:
                                nc.vector.tensor_tensor(
                                    out=S[:],
                                    in0=Xo[:, 4 - d:4 - d + W],
                                    in1=Xo[:, 4 + d:4 + d + W],
                                    op=mybir.AluOpType.add)
                            else:
                                nc.vector.tensor_tensor(
                                    out=S[:], in0=X[:, HALO - d:HALO - d + W],
                                    in1=X[:, HALO + d:HALO + d + W],
                                    op=mybir.AluOpType.add)
                            svec[d] = S

                    thr_t = thrp.tile([TH, W], F32, tag="thr")
                    nc.sync.dma_start(out=thr_t[:], in_=thr_d[l, r0:r0 + TH, :])

                    # contrib planes for connections with src == l (dst > l)
                    for ci in by_src.get(l, []):
                        wt = wp.tile([KR, 2 * W], BF16, tag="w")
                        nc.scalar.dma_start(out=wt[:],
                                            in_=wpk_d[ci, r0:r0 + KR, :])
                        xs = Xo[:, 4:4 + W]
                        chi = cp.tile([KR, W], BF16, tag="c")
                        clo = cp.tile([KR, W], BF16, tag="c")
                        nc.vector.tensor_tensor(out=chi[:], in0=xs,
                                                in1=wt[:, 0:W],
                                                op=mybir.AluOpType.mult)
                        nc.vector.tensor_tensor(out=clo[:], in0=xs,
                                                in1=wt[:, W:2 * W],
                                                op=mybir.AluOpType.mult)
                        contrib[ci] = (chi, clo)

                    flush_pending()

                    out_t = op.tile([TH, W], BF16, tag="out")
                    my_contribs = [contrib[ci] for ci in by_dst.get(l, [])]
                    ps = psp.tile([TH, W], F32)  # 3 PSUM banks

                    for n in range(NT):
                        c0 = n * NFREE
                        n_mm = 2 * NG + 2 * len(my_contribs)
                        mm = 0
                        order = []
                        for gi, (kind, d) in enumerate(groups_meta):
                            order.append((0, gi, kind, d))
                            order.append((1, gi, kind, d))
                        for part, gi, kind, d in order:
                            lhsT = bands_sb[:, (part * NG + gi) * TH:
                                            (part * NG + gi + 1) * TH]
                            if kind == "pair" and d > 0:
                                rhs = svec[d][:, c0:c0 + NFREE]
                            else:
                                dx = 0 if kind == "pair" else d
                                rhs = X[:, HALO + dx + c0:
                                        HALO + dx + c0 + NFREE]
                            nc.tensor.matmul(ps[:, c0:c0 + NFREE], lhsT, rhs,
                                             start=(mm == 0),
                                             stop=(mm == n_mm - 1))
                            mm += 1
                        for chi, clo in my_contribs:
                            for ct in (chi, clo):
                                nc.tensor.matmul(ps[:, c0:c0 + NFREE],
                                                 iden_sb[:],
                                                 ct[:, c0:c0 + NFREE],
                                                 start=(mm == 0),
                                                 stop=(mm == n_mm - 1))
                                mm += 1
                    pending[0] = (ps, thr_t, out_t, l, r0)
            flush_pending()

    nc.compile()
    return nc


_PROGRAM_CACHE = {}


def _get_program(conns, R, W, groups_meta):
    key = (tuple(conns), R, W, tuple(groups_meta))
    if key not in _PROGRAM_CACHE:
        _PROGRAM_CACHE[key] = _build_program(conns, R, W, groups_meta)
    return _PROGRAM_CACHE[key]


def _prepare_inputs(external, prev_spikes, membrane, inter_weights,
                    local_kernel, refractory, conn_src, conn_dst):
    Lx, H, W = external.shape
    R = H // NCORES
    conns = [(int(s), int(d)) for s, d in zip(conn_src, conn_dst)]

    groups = _group_kernel_columns(np.asarray(local_kernel, np.float32))
    groups_meta = [(k, d) for k, d, _c in groups]

    # band matrices, hi parts then lo parts, [KR, 2*NG*TH] bf16
    NG = len(groups)
    bands = np.zeros((KR, 2 * NG * TH), ml_dtypes.bfloat16)
    for gi, (_k, _d, col) in enumerate(groups):
        B = _band_matrix(col)
        hi, lo = _split_bf16(B)
        bands[:, gi * TH:(gi + 1) * TH] = hi
        bands[:, (NG + gi) * TH:(NG + gi + 1) * TH] = lo
    # shifted identity: psum row m accumulates contrib tile row m+5
    iden = np.zeros((KR, TH), ml_dtypes.bfloat16)
    for m in range(TH):
        iden[m + HALO, m] = 1.0

    # fp32 threshold plane: out fires iff psum > thr
    ext = np.asarray(external, np.float32)
    mem = np.asarray(membrane, np.float32)
    refr = np.asarray(refractory)
    thr = (BIG * (refr != 0).astype(np.float32)
           - (ext + DECAY * mem)).astype(np.float32)

    # padded bf16 spikes (exact: values {0,1}); 5 left / 7 right columns
    spk = np.zeros((Lx, H + 2 * HALO, W + WPAD), ml_dtypes.bfloat16)
    spk[:, HALO:H + HALO, HALO:W + HALO] = np.asarray(prev_spikes, np.float32)

    w_hi, w_lo = _split_bf16(np.asarray(inter_weights, np.float32))
    wpk = np.zeros((len(conns), H + 2 * HALO, 2 * W), ml_dtypes.bfloat16)
    wpk[:, HALO:H + HALO, 0:W] = w_hi
    wpk[:, HALO:H + HALO, W:2 * W] = w_lo

    in_maps = []
    for c in range(NCORES):
        g0 = c * R
        in_maps.append({
            "spk": np.ascontiguousarray(spk[:, g0:g0 + R + 2 * HALO, :]),
            "thr": np.ascontiguousarray(thr[:, g0:g0 + R, :]),
            "wpk": np.ascontiguousarray(wpk[:, g0:g0 + R + 2 * HALO, :]),
            "bands": bands,
            "iden": iden,
        })
    return conns, R, W, groups_meta, in_maps


def _ensure_ntff_hook():
    """Inject the missing antenv.axon_hooks module + ctypes NTFF hook so
    trace=True works in this image (profiling only; best-effort)."""
    import types
    try:
        import antenv.axon_hooks  # noqa: F401
        return
    except ImportError:
        pass
    try:
        import antenv
        mod = types.ModuleType("antenv.axon_hooks")
        _h = [None]
        mod.set_axon_ntff_profile_hook = lambda h: _h.__setitem__(0, h)
        mod.get_axon_ntff_profile_hook = lambda: _h[0]
        sys.modules["antenv.axon_hooks"] = mod
        antenv.axon_hooks = mod
        from trn_agent_boot.trn_boot import _ntff_profile_via_ctypes
        hook = _ntff_profile_via_ctypes("/opt/axon/libaxon_pjrt.so")
        if hook is not None:
            _h[0] = hook
    except Exception:
        pass


def kernel(external, prev_spikes, membrane, inter_weights, local_kernel,
           refractory, conn_src, conn_dst, _trace=False):
    if _trace:
        _ensure_ntff_hook()
    conns, R, W, groups_meta, in_maps = _prepare_inputs(
        external, prev_spikes, membrane, inter_weights, local_kernel,
        refractory, conn_src, conn_dst)
    nc = _get_program(conns, R, W, groups_meta)
    res = run_bass_kernel_spmd(nc, in_maps, core_ids=list(range(NCORES)),
                               trace=_trace)
    out = np.concatenate([r["out"].astype(np.float32) for r in res.results],
                         axis=1)
    if _trace:
        kernel._last_results = res
    return out


# revision 21
# speedup vs baseline: 1.8788x; 1.8788x over previous
"""Trainium2 Bass kernel for nn_CognitiveModule (gnn_message_passing).

Computes, for L=8 layers of a 1536x1536 grid:
  internal = conv2d(prev_spikes, local_kernel, SAME)      # 11x11 distance kernel
  axonal   = segment_sum(prev_spikes[conn_src] * inter_weights, conn_dst)
  total    = external + internal + axonal
  active   = (refractory == 0)
  v_new    = 0.9 * membrane + active * total
  spikes   = (v_new > 0) * active          (the sigmoid straight-through term
                                            cancels in the forward pass)

Strategy (8 NeuronCores, shard H):
  - Each core gets 192 rows of every layer (plus a 5-row halo of prev_spikes).
  - Conv runs on the TensorEngine as banded matmuls over the row (partition)
    dimension: for each kernel column kx, a [106,96] band matrix contracts 106
    input rows into 96 output rows.  The x-taps are reduced from 11 to 6
    matmul passes via the kernel's x-symmetry: the DVE pre-adds the shifted
    spike images (S_d = X_{-d} + X_{+d}; spikes are {0,1} so sums are exact
    in bf16).  A one-column-shifted copy (Xo) keeps every DVE op 4B-aligned
    (2x perf mode).
  - All matmul data is bf16.  Kernel/weight values are split hi/lo
    (w = bf16(w) + bf16(w - bf16(w))) so products against {0,1,2} spikes are
    exact to ~2^-18 relative - fp32-class accuracy at bf16 matmul speed.
  - external + 0.9*membrane and the refractory gate are folded on the host
    into one fp32 threshold plane  thr = BIG*(refr != 0) - (ext + 0.9*mem);
    the device finalize is ONE VectorEngine op per row-block:
        out = (psum > thr)  as {1.0, 0.0}  (bf16, cast to fp32 on the host).
  - Axonal products are computed on the VectorEngine from the resident spike
    tiles (over all 106 rows) and accumulated into PSUM with a shifted
    identity matmul (iden5: rows 5..100 -> psum rows 0..95).  Connections
    with src >= dst get a dedicated spike load at the start of each
    row-block.  The finalize is software-pipelined one layer behind the
    matmuls so the PE never waits on the DVE queue.
"""

import sys

for _p in ("/opt/trn_rl_repo", "/root/.axon_site/_ro/trn_rl_repo"):
    if _p not in sys.path:
        sys.path.append(_p)

import numpy as np
import ml_dtypes

import concourse.bass as bass
import concourse.mybir as mybir
import concourse.tile as tile
from concourse import bacc
from concourse.bass_utils import run_bass_kernel_spmd

BF16 = mybir.dt.bfloat16
F32 = mybir.dt.float32
BIG = np.float32(1.0e5)
DECAY = np.float32(0.9)

L = 8
NCORES = 8
TH = 96          # output rows per conv tile
HALO = 5
KS = 11          # kernel size
KR = TH + 2 * HALO  # 106 input rows per conv tile
NFREE = 512      # psum free-dim tile
WPAD = 12        # spk row padding: 5 left + 7 right (even widths, alignment)


def _split_bf16(x):
    hi = x.astype(ml_dtypes.bfloat16)
    lo = (x - hi.astype(np.float32)).astype(ml_dtypes.bfloat16)
    return hi, lo


def _group_kernel_columns(kern):
    """Group the 11 kernel columns by x-symmetry: ("pair", d) groups read the
    pre-added S_d image; ("single", dx) groups read a shifted X window."""
    groups = []
    used = [False] * KS
    for d in range(0, HALO + 1):
        a, b = HALO + d, HALO - d
        if d == 0:
            groups.append(("pair", 0, kern[:, HALO].copy()))
            used[HALO] = True
        elif np.array_equal(kern[:, a], kern[:, b]):
            groups.append(("pair", d, kern[:, a].copy()))
            used[a] = used[b] = True
    for kx in range(KS):
        if not used[kx]:
            groups.append(("single", kx - HALO, kern[:, kx].copy()))
    return groups


def _band_matrix(col):
    """[KR, TH] band matrix: B[k, m] = col[k - m] for 0 <= k-m <= 10.
    X partition k holds spike row r0 + k - 5 (straight layout)."""
    B = np.zeros((KR, TH), np.float32)
    for m in range(TH):
        for ky in range(KS):
            B[m + ky, m] = col[ky]
    return B


def _build_program(conns, R, W, groups_meta):
    """Build the SPMD Bass program (identical on all cores)."""
    nc = bacc.Bacc(None, target_bir_lowering=False, debug=False)
    NT = W // NFREE
    HT = R // TH
    NG = len(groups_meta)

    spk_d = nc.dram_tensor("spk", [L, R + 2 * HALO, W + WPAD], BF16,
                           kind="ExternalInput")
    thr_d = nc.dram_tensor("thr", [L, R, W], F32, kind="ExternalInput")
    # w hi/lo packed side by side, rows padded by the conv halo:
    # [C, 5 + r + 5, 0:W] = hi(row r), [.., W:2W] = lo(row r)
    wpk_d = nc.dram_tensor("wpk", [len(conns), R + 2 * HALO, 2 * W], BF16,
                           kind="ExternalInput")
    bands_d = nc.dram_tensor("bands", [KR, 2 * NG * TH], BF16,
                             kind="ExternalInput")
    iden_d = nc.dram_tensor("iden", [KR, TH], BF16, kind="ExternalInput")
    out_d = nc.dram_tensor("out", [L, R, W], BF16, kind="ExternalOutput")

    pre_conns = [i for i, (s, d) in enumerate(conns) if s >= d]
    inline_conns = [i for i, (s, d) in enumerate(conns) if s < d]
    by_src = {}
    for i in inline_conns:
        by_src.setdefault(conns[i][0], []).append(i)
    by_dst = {}
    for i in range(len(conns)):
        by_dst.setdefault(conns[i][1], []).append(i)

    with tile.TileContext(nc) as tc:
        with (
            tc.tile_pool(name="const", bufs=1) as constp,
            tc.tile_pool(name="xp", bufs=3) as xp,
            tc.tile_pool(name="sp", bufs=2) as sp,
            tc.tile_pool(name="thrp", bufs=3) as thrp,
            tc.tile_pool(name="wp", bufs=6) as wp,
            tc.tile_pool(name="cp", bufs=12) as cp,
            tc.tile_pool(name="op", bufs=3) as op,
            tc.tile_pool(name="prep", bufs=2) as prep,
            tc.tile_pool(name="ps", bufs=2, space="PSUM") as psp,
        ):
            bands_sb = constp.tile([KR, 2 * NG * TH], BF16)
            nc.sync.dma_start(out=bands_sb[:], in_=bands_d[:])
            iden_sb = constp.tile([KR, TH], BF16)
            nc.sync.dma_start(out=iden_sb[:], in_=iden_d[:])

            # deferred finalize, one layer behind (PE never waits on DVE)
            pending = [None]

            def flush_pending():
                if pending[0] is None:
                    return
                ps_p, thr_p, out_p, l_p, r0_p = pending[0]
                nc.vector.tensor_tensor(
                    out=out_p[:], in0=ps_p[:], in1=thr_p[:],
                    op=mybir.AluOpType.is_gt)
                nc.gpsimd.dma_start(out=out_d[l_p, r0_p:r0_p + TH, :],
                                    in_=out_p[:])
                pending[0] = None

            for h in range(HT):
                r0 = h * TH
                contrib = {}  # conn idx -> (hi_tile, lo_tile)

                # connections whose src comes later in the layer loop: load
                # the needed spike rows now (odd-aligned, all 106 rows).
                for ci in pre_conns:
                    s = conns[ci][0]
                    spre = prep.tile([KR, W + 8], BF16, tag="spre")
                    nc.gpsimd.dma_start(
                        out=spre[:],
                        in_=spk_d[s, r0:r0 + KR, 1:W + 9])
                    wt = wp.tile([KR, 2 * W], BF16, tag="w")
                    nc.gpsimd.dma_start(out=wt[:],
                                        in_=wpk_d[ci, r0:r0 + KR, :])
                    chi = cp.tile([KR, W], BF16, tag="c")
                    clo = cp.tile([KR, W], BF16, tag="c")
                    xs = spre[:, 4:4 + W]
                    nc.vector.tensor_tensor(out=chi[:], in0=xs,
                                            in1=wt[:, 0:W],
                                            op=mybir.AluOpType.mult)
                    nc.vector.tensor_tensor(out=clo[:], in0=xs,
                                            in1=wt[:, W:2 * W],
                                            op=mybir.AluOpType.mult)
                    contrib[ci] = (chi, clo)

                for l in range(L):
                    # X[k] = spike row r0 + k - 5 (dram row r0 + k), cols at
                    # byte 2x.  Xo is shifted one column (odd alignment) for
                    # the even-d pre-adds and the contrib muls.
                    X = xp.tile([KR, W + 2 * HALO], BF16, tag="X")
                    nc.gpsimd.dma_start(
                        out=X[:], in_=spk_d[l, r0:r0 + KR, 0:W + 2 * HALO])
                    Xo = xp.tile([KR, W + 8], BF16, tag="Xo")
                    nc.gpsimd.dma_start(
                        out=Xo[:], in_=spk_d[l, r0:r0 + KR, 1:W + 9])

                    # symmetric pre-adds S_d = X_{-d} + X_{+d} (all DVE 2x)
                    svec = {}
                    for gi, (kind, d) in enumerate(groups_meta):
                        if kind == "pair" and d > 0:
                            S = sp.tile([KR, W], BF16, tag=f"S{d}")
                            if d % 2 == 0:
                                nc.vector.tensor_tensor(
                                    out=S[:],
                                    in0=Xo[:, 4 - d:4 - d + W],
                                    in1=Xo[:, 4 + d:4 + d + W],
                                    op=mybir.AluOpType.add)
                            else:
                                nc.vector.tensor_tensor(
                                    out=S[:], in0=X[:, HALO - d:HALO - d + W],
                                    in1=X[:, HALO + d:HALO + d + W],
                                    op=mybir.AluOpType.add)
                            svec[d] = S

                    thr_t = thrp.tile([TH, W], F32, tag="thr")
                    nc.gpsimd.dma_start(out=thr_t[:],
                                        in_=thr_d[l, r0:r0 + TH, :])

                    # contrib planes for connections with src == l (dst > l)
                    for ci in by_src.get(l, []):
                        wt = wp.tile([KR, 2 * W], BF16, tag="w")
                        nc.gpsimd.dma_start(out=wt[:],
                                            in_=wpk_d[ci, r0:r0 + KR, :])
                        xs = Xo[:, 4:4 + W]
                        chi = cp.tile([KR, W], BF16, tag="c")
                        clo = cp.tile([KR, W], BF16, tag="c")
                        nc.vector.tensor_tensor(out=chi[:], in0=xs,
                                                in1=wt[:, 0:W],
                                                op=mybir.AluOpType.mult)
                        nc.vector.tensor_tensor(out=clo[:], in0=xs,
                                                in1=wt[:, W:2 * W],
                                                op=mybir.AluOpType.mult)
                        contrib[ci] = (chi, clo)

                    flush_pending()

                    out_t = op.tile([TH, W], BF16, tag="out")
                    my_contribs = [contrib[ci] for ci in by_dst.get(l, [])]
                    ps = psp.tile([TH, W], F32)  # 3 PSUM banks

                    for n in range(NT):
                        c0 = n * NFREE
                        n_mm = 2 * NG + 2 * len(my_contribs)
                        mm = 0
                        order = []
                        for gi, (kind, d) in enumerate(groups_meta):
                            order.append((0, gi, kind, d))
                            order.append((1, gi, kind, d))
                        for part, gi, kind, d in order:
                            lhsT = bands_sb[:, (part * NG + gi) * TH:
                                            (part * NG + gi + 1) * TH]
                            if kind == "pair" and d > 0:
                                rhs = svec[d][:, c0:c0 + NFREE]
                            else:
                                dx = 0 if kind == "pair" else d
                                rhs = X[:, HALO + dx + c0:
                                        HALO + dx + c0 + NFREE]
                            nc.tensor.matmul(ps[:, c0:c0 + NFREE], lhsT, rhs,
                                             start=(mm == 0),
                                             stop=(mm == n_mm - 1))
                            mm += 1
                        for chi, clo in my_contribs:
                            for ct in (chi, clo):
                                nc.tensor.matmul(ps[:, c0:c0 + NFREE],
                                                 iden_sb[:],
                                                 ct[:, c0:c0 + NFREE],
                                                 start=(mm == 0),
                                                 stop=(mm == n_mm - 1))
                                mm += 1
                    pending[0] = (ps, thr_t, out_t, l, r0)
            flush_pending()

    nc.compile()
    return nc


_PROGRAM_CACHE = {}


def _get_program(conns, R, W, groups_meta):
    key = (tuple(conns), R, W, tuple(groups_meta))
    if key not in _PROGRAM_CACHE:
        _PROGRAM_CACHE[key] = _build_program(conns, R, W, groups_meta)
    return _PROGRAM_CACHE[key]


def _prepare_inputs(external, prev_spikes, membrane, inter_weights,
                    local_kernel, refractory, conn_src, conn_dst):
    Lx, H, W = external.shape
    R = H // NCORES
    conns = [(int(s), int(d)) for s, d in zip(conn_src, conn_dst)]

    groups = _group_kernel_columns(np.asarray(local_kernel, np.float32))
    groups_meta = [(k, d) for k, d, _c in groups]

    # band matrices, hi parts then lo parts, [KR, 2*NG*TH] bf16
    NG = len(groups)
    bands = np.zeros((KR, 2 * NG * TH), ml_dtypes.bfloat16)
    for gi, (_k, _d, col) in enumerate(groups):
        B = _band_matrix(col)
        hi, lo = _split_bf16(B)
        bands[:, gi * TH:(gi + 1) * TH] = hi
        bands[:, (NG + gi) * TH:(NG + gi + 1) * TH] = lo
    # shifted identity: psum row m accumulates contrib tile row m+5
    iden = np.zeros((KR, TH), ml_dtypes.bfloat16)
    for m in range(TH):
        iden[m + HALO, m] = 1.0

    # fp32 threshold plane: out fires iff psum > thr
    ext = np.asarray(external, np.float32)
    mem = np.asarray(membrane, np.float32)
    refr = np.asarray(refractory)
    thr = (BIG * (refr != 0).astype(np.float32)
           - (ext + DECAY * mem)).astype(np.float32)

    # padded bf16 spikes (exact: values {0,1}); 5 left / 7 right columns
    spk = np.zeros((Lx, H + 2 * HALO, W + WPAD), ml_dtypes.bfloat16)
    spk[:, HALO:H + HALO, HALO:W + HALO] = np.asarray(prev_spikes, np.float32)

    w_hi, w_lo = _split_bf16(np.asarray(inter_weights, np.float32))
    wpk = np.zeros((len(conns), H + 2 * HALO, 2 * W), ml_dtypes.bfloat16)
    wpk[:, HALO:H + HALO, 0:W] = w_hi
    wpk[:, HALO:H + HALO, W:2 * W] = w_lo

    in_maps = []
    for c in range(NCORES):
        g0 = c * R
        in_maps.append({
            "spk": np.ascontiguousarray(spk[:, g0:g0 + R + 2 * HALO, :]),
            "thr": np.ascontiguousarray(thr[:, g0:g0 + R, :]),
            "wpk": np.ascontiguousarray(wpk[:, g0:g0 + R + 2 * HALO, :]),
            "bands": bands,
            "iden": iden,
        })
    return conns, R, W, groups_meta, in_maps


def _ensure_ntff_hook():
    """Inject the missing antenv.axon_hooks module + ctypes NTFF hook so
    trace=True works in this image (profiling only; best-effort)."""
    import types
    try:
        import antenv.axon_hooks  # noqa: F401
        return
    except ImportError:
        pass
    try:
        import antenv
        mod = types.ModuleType("antenv.axon_hooks")
        _h = [None]
        mod.set_axon_ntff_profile_hook = lambda h: _h.__setitem__(0, h)
        mod.get_axon_ntff_profile_hook = lambda: _h[0]
        sys.modules["antenv.axon_hooks"] = mod
        antenv.axon_hooks = mod
        from trn_agent_boot.trn_boot import _ntff_profile_via_ctypes
        hook = _ntff_profile_via_ctypes("/opt/axon/libaxon_pjrt.so")
        if hook is not None:
            _h[0] = hook
    except Exception:
        pass


def kernel(external, prev_spikes, membrane, inter_weights, local_kernel,
           refractory, conn_src, conn_dst, _trace=False):
    if _trace:
        _ensure_ntff_hook()
    conns, R, W, groups_meta, in_maps = _prepare_inputs(
        external, prev_spikes, membrane, inter_weights, local_kernel,
        refractory, conn_src, conn_dst)
    nc = _get_program(conns, R, W, groups_meta)
    res = run_bass_kernel_spmd(nc, in_maps, core_ids=list(range(NCORES)),
                               trace=_trace)
    out = np.concatenate([r["out"].astype(np.float32) for r in res.results],
                         axis=1)
    if _trace:
        kernel._last_results = res
    return out


# revision 22
# speedup vs baseline: 2.1983x; 1.1700x over previous
"""Trainium2 Bass kernel for nn_CognitiveModule (gnn_message_passing).

Computes, for L=8 layers of a 1536x1536 grid:
  internal = conv2d(prev_spikes, local_kernel, SAME)      # 11x11 distance kernel
  axonal   = segment_sum(prev_spikes[conn_src] * inter_weights, conn_dst)
  total    = external + internal + axonal
  active   = (refractory == 0)
  v_new    = 0.9 * membrane + active * total
  spikes   = (v_new > 0) * active          (the sigmoid straight-through term
                                            cancels in the forward pass)

Strategy (8 NeuronCores, shard H):
  - Each core gets 192 rows of every layer (plus a 5-row halo of prev_spikes).
  - Conv runs on the TensorEngine as banded matmuls over the row (partition)
    dimension: for each kernel column kx, a [106,96] band matrix contracts 106
    input rows into 96 output rows.  The x-taps are reduced from 11 to 6
    matmul passes via the kernel's x-symmetry: the DVE pre-adds the shifted
    spike images (S_d = X_{-d} + X_{+d}; spikes are {0,1} so sums are exact
    in bf16).  A one-column-shifted copy (Xo) keeps every DVE op 4B-aligned
    (2x perf mode).
  - All matmul data is bf16.  Kernel/weight values are split hi/lo
    (w = bf16(w) + bf16(w - bf16(w))) so products against {0,1,2} spikes are
    exact to ~2^-18 relative - fp32-class accuracy at bf16 matmul speed.
  - external + 0.9*membrane and the refractory gate are folded on the host
    into one fp32 threshold plane  thr = BIG*(refr != 0) - (ext + 0.9*mem);
    the device finalize is ONE VectorEngine op per row-block:
        out = (psum > thr)  as {1.0, 0.0}  (bf16, cast to fp32 on the host).
  - Axonal products are computed on the VectorEngine from the resident spike
    tiles (over all 106 rows) and accumulated into PSUM with a shifted
    identity matmul (iden5: rows 5..100 -> psum rows 0..95).  Connections
    with src >= dst get a dedicated spike load at the start of each
    row-block.  The finalize is software-pipelined one layer behind the
    matmuls so the PE never waits on the DVE queue.
"""

import sys

for _p in ("/opt/trn_rl_repo", "/root/.axon_site/_ro/trn_rl_repo"):
    if _p not in sys.path:
        sys.path.append(_p)

import numpy as np
import ml_dtypes

import dataclasses

import concourse.bass as bass
import concourse.mybir as mybir
import concourse.tile as tile
from concourse import bacc
from concourse.bass_utils import run_bass_kernel_spmd

DT16 = mybir.dt.float16
NP16 = np.float16
F32 = mybir.dt.float32
BIG = np.float32(1.0e5)
DECAY = np.float32(0.9)

L = 8
NCORES = 8
TH = 96          # output rows per conv tile
HALO = 5
KS = 11          # kernel size
KR = TH + 2 * HALO  # 106 input rows per conv tile
NFREE = 512      # psum free-dim tile
WPAD = 12        # spk row padding: 5 left + 7 right (even widths, alignment)


def _split16(x):
    hi = x.astype(NP16)
    lo = (x - hi.astype(np.float32)).astype(NP16)
    return hi, lo


def _group_kernel_columns(kern):
    """Group the 11 kernel columns by x-symmetry: ("pair", d) groups read the
    pre-added S_d image; ("single", dx) groups read a shifted X window."""
    groups = []
    used = [False] * KS
    for d in range(0, HALO + 1):
        a, b = HALO + d, HALO - d
        if d == 0:
            groups.append(("pair", 0, kern[:, HALO].copy()))
            used[HALO] = True
        elif np.array_equal(kern[:, a], kern[:, b]):
            groups.append(("pair", d, kern[:, a].copy()))
            used[a] = used[b] = True
    for kx in range(KS):
        if not used[kx]:
            groups.append(("single", kx - HALO, kern[:, kx].copy()))
    return groups


def _band_matrix(col):
    """[KR, TH] band matrix: B[k, m] = col[k - m] for 0 <= k-m <= 10.
    X partition k holds spike row r0 + k - 5 (straight layout)."""
    B = np.zeros((KR, TH), np.float32)
    for m in range(TH):
        for ky in range(KS):
            B[m + ky, m] = col[ky]
    return B


def _build_program(conns, R, W, groups_meta):
    """Build the SPMD Bass program (identical on all cores)."""
    nc = bacc.Bacc(None, target_bir_lowering=False, debug=False)
    NT = W // NFREE
    HT = R // TH
    NG = len(groups_meta)

    spk_d = nc.dram_tensor("spk", [L, R + 2 * HALO, W + WPAD], DT16,
                           kind="ExternalInput")
    thr_d = nc.dram_tensor("thr", [L, R, W], F32, kind="ExternalInput")
    # single-fp16 weights, rows padded by the conv halo
    wpk_d = nc.dram_tensor("wpk", [len(conns), R + 2 * HALO, W], DT16,
                           kind="ExternalInput")
    bands_d = nc.dram_tensor("bands", [KR, 2 * NG * TH], DT16,
                             kind="ExternalInput")
    iden_d = nc.dram_tensor("iden", [KR, TH], DT16, kind="ExternalInput")
    out_d = nc.dram_tensor("out", [L, R, W], DT16, kind="ExternalOutput")

    pre_conns = [i for i, (s, d) in enumerate(conns) if s >= d]
    inline_conns = [i for i, (s, d) in enumerate(conns) if s < d]
    by_src = {}
    for i in inline_conns:
        by_src.setdefault(conns[i][0], []).append(i)
    by_dst = {}
    for i in range(len(conns)):
        by_dst.setdefault(conns[i][1], []).append(i)

    with tile.TileContext(nc) as tc:
        with (
            tc.tile_pool(name="const", bufs=1) as constp,
            tc.tile_pool(name="xp", bufs=3) as xp,
            tc.tile_pool(name="sp", bufs=2) as sp,
            tc.tile_pool(name="thrp", bufs=3) as thrp,
            tc.tile_pool(name="wp", bufs=6) as wp,
            tc.tile_pool(name="cp", bufs=12) as cp,
            tc.tile_pool(name="op", bufs=3) as op,
            tc.tile_pool(name="prep", bufs=2) as prep,
            tc.tile_pool(name="ps", bufs=2, space="PSUM") as psp,
        ):
            bands_sb = constp.tile([KR, 2 * NG * TH], DT16)
            nc.sync.dma_start(out=bands_sb[:], in_=bands_d[:])
            iden_sb = constp.tile([KR, TH], DT16)
            nc.sync.dma_start(out=iden_sb[:], in_=iden_d[:])

            # deferred finalize, one layer behind (PE never waits on DVE)
            pending = [None]

            def flush_pending():
                if pending[0] is None:
                    return
                ps_p, thr_p, out_p, l_p, r0_p = pending[0]
                nc.vector.tensor_tensor(
                    out=out_p[:], in0=ps_p[:], in1=thr_p[:],
                    op=mybir.AluOpType.is_gt)
                nc.gpsimd.dma_start(out=out_d[l_p, r0_p:r0_p + TH, :],
                                    in_=out_p[:])
                pending[0] = None

            for h in range(HT):
                r0 = h * TH
                contrib = {}  # conn idx -> (hi_tile, lo_tile)

                # connections whose src comes later in the layer loop: load
                # the needed spike rows now (odd-aligned, all 106 rows).
                for ci in pre_conns:
                    s = conns[ci][0]
                    spre = prep.tile([KR, W + 8], DT16, tag="spre")
                    nc.gpsimd.dma_start(
                        out=spre[:],
                        in_=spk_d[s, r0:r0 + KR, 1:W + 9])
                    wt = wp.tile([KR, W], DT16, tag="w")
                    nc.gpsimd.dma_start(out=wt[:],
                                        in_=wpk_d[ci, r0:r0 + KR, :])
                    chi = cp.tile([KR, W], DT16, tag="c")
                    nc.vector.tensor_tensor(out=chi[:], in0=spre[:, 4:4 + W],
                                            in1=wt[:],
                                            op=mybir.AluOpType.mult)
                    contrib[ci] = (chi,)

                for l in range(L):
                    # X[k] = spike row r0 + k - 5 (dram row r0 + k), cols at
                    # byte 2x.  Xo is shifted one column (odd alignment) for
                    # the even-d pre-adds and the contrib muls.
                    # one DMA loads both the aligned (X) and one-column-
                    # shifted (Xo) copies: src has an overlapping [1,2] dim.
                    XX = xp.tile([KR, 2 * (W + WPAD)], DT16, tag="X")
                    src = spk_d[l, r0:r0 + KR, 0:W + WPAD - 1]
                    src = dataclasses.replace(
                        src, ap=[[W + WPAD, KR], [1, 2], [1, W + WPAD - 1]])
                    dst = XX[:].rearrange("p (j x) -> p j x", x=W + WPAD)
                    nc.gpsimd.dma_start(out=dst[:, :, 0:W + WPAD - 1], in_=src)
                    X = XX[:, 0:W + WPAD]
                    Xo = XX[:, W + WPAD:2 * (W + WPAD)]

                    # symmetric pre-adds S_d = X_{-d} + X_{+d} (all DVE 2x)
                    svec = {}
                    for gi, (kind, d) in enumerate(groups_meta):
                        if kind == "pair" and d > 0:
                            S = sp.tile([KR, W], DT16, tag=f"S{d}")
                            if d % 2 == 0:
                                nc.vector.tensor_tensor(
                                    out=S[:],
                                    in0=Xo[:, 4 - d:4 - d + W],
                                    in1=Xo[:, 4 + d:4 + d + W],
                                    op=mybir.AluOpType.add)
                            else:
                                nc.vector.tensor_tensor(
                                    out=S[:], in0=X[:, HALO - d:HALO - d + W],
                                    in1=X[:, HALO + d:HALO + d + W],
                                    op=mybir.AluOpType.add)
                            svec[d] = S

                    thr_t = thrp.tile([TH, W], F32, tag="thr")
                    nc.gpsimd.dma_start(out=thr_t[:],
                                        in_=thr_d[l, r0:r0 + TH, :])

                    # contrib planes for connections with src == l (dst > l)
                    for ci in by_src.get(l, []):
                        wt = wp.tile([KR, W], DT16, tag="w")
                        nc.gpsimd.dma_start(out=wt[:],
                                            in_=wpk_d[ci, r0:r0 + KR, :])
                        chi = cp.tile([KR, W], DT16, tag="c")
                        nc.vector.tensor_tensor(out=chi[:], in0=Xo[:, 4:4 + W],
                                                in1=wt[:],
                                                op=mybir.AluOpType.mult)
                        contrib[ci] = (chi,)

                    flush_pending()

                    out_t = op.tile([TH, W], DT16, tag="out")
                    my_contribs = [contrib[ci] for ci in by_dst.get(l, [])]
                    ps = psp.tile([TH, W], F32)  # 3 PSUM banks

                    for n in range(NT):
                        c0 = n * NFREE
                        n_mm = 2 * NG + sum(len(t) for t in my_contribs)
                        mm = 0
                        order = []
                        for gi, (kind, d) in enumerate(groups_meta):
                            order.append((0, gi, kind, d))
                            order.append((1, gi, kind, d))
                        for part, gi, kind, d in order:
                            lhsT = bands_sb[:, (part * NG + gi) * TH:
                                            (part * NG + gi + 1) * TH]
                            if kind == "pair" and d > 0:
                                rhs = svec[d][:, c0:c0 + NFREE]
                            else:
                                dx = 0 if kind == "pair" else d
                                rhs = X[:, HALO + dx + c0:
                                        HALO + dx + c0 + NFREE]
                            nc.tensor.matmul(ps[:, c0:c0 + NFREE], lhsT, rhs,
                                             start=(mm == 0),
                                             stop=(mm == n_mm - 1))
                            mm += 1
                        for cts in my_contribs:
                            for ct in cts:
                                nc.tensor.matmul(ps[:, c0:c0 + NFREE],
                                                 iden_sb[:],
                                                 ct[:, c0:c0 + NFREE],
                                                 start=(mm == 0),
                                                 stop=(mm == n_mm - 1))
                                mm += 1
                    pending[0] = (ps, thr_t, out_t, l, r0)
            flush_pending()

    nc.compile()
    return nc


_PROGRAM_CACHE = {}


def _get_program(conns, R, W, groups_meta):
    key = (tuple(conns), R, W, tuple(groups_meta))
    if key not in _PROGRAM_CACHE:
        _PROGRAM_CACHE[key] = _build_program(conns, R, W, groups_meta)
    return _PROGRAM_CACHE[key]


def _prepare_inputs(external, prev_spikes, membrane, inter_weights,
                    local_kernel, refractory, conn_src, conn_dst):
    Lx, H, W = external.shape
    R = H // NCORES
    conns = [(int(s), int(d)) for s, d in zip(conn_src, conn_dst)]

    groups = _group_kernel_columns(np.asarray(local_kernel, np.float32))
    groups_meta = [(k, d) for k, d, _c in groups]

    # band matrices, hi parts then lo parts, [KR, 2*NG*TH] bf16
    NG = len(groups)
    bands = np.zeros((KR, 2 * NG * TH), NP16)
    for gi, (_k, _d, col) in enumerate(groups):
        B = _band_matrix(col)
        hi, lo = _split16(B)
        bands[:, gi * TH:(gi + 1) * TH] = hi
        bands[:, (NG + gi) * TH:(NG + gi + 1) * TH] = lo
    # shifted identity: psum row m accumulates contrib tile row m+5
    iden = np.zeros((KR, TH), NP16)
    for m in range(TH):
        iden[m + HALO, m] = 1.0

    # fp32 threshold plane: out fires iff psum > thr
    ext = np.asarray(external, np.float32)
    mem = np.asarray(membrane, np.float32)
    refr = np.asarray(refractory)
    thr = (BIG * (refr != 0).astype(np.float32)
           - (ext + DECAY * mem)).astype(np.float32)

    # padded fp16 spikes (exact: values {0,1}); 5 left / 7 right columns
    spk = np.zeros((Lx, H + 2 * HALO, W + WPAD), NP16)
    spk[:, HALO:H + HALO, HALO:W + HALO] = np.asarray(prev_spikes, np.float32)

    # single fp16 weights (products vs {0,1} spikes are exact; the only
    # error is the ~2^-11 fp16 quantization of w itself)
    wpk = np.zeros((len(conns), H + 2 * HALO, W), NP16)
    wpk[:, HALO:H + HALO, :] = np.asarray(inter_weights, np.float32)

    in_maps = []
    for c in range(NCORES):
        g0 = c * R
        in_maps.append({
            "spk": np.ascontiguousarray(spk[:, g0:g0 + R + 2 * HALO, :]),
            "thr": np.ascontiguousarray(thr[:, g0:g0 + R, :]),
            "wpk": np.ascontiguousarray(wpk[:, g0:g0 + R + 2 * HALO, :]),
            "bands": bands,
            "iden": iden,
        })
    return conns, R, W, groups_meta, in_maps


def _ensure_ntff_hook():
    """Inject the missing antenv.axon_hooks module + ctypes NTFF hook so
    trace=True works in this image (profiling only; best-effort)."""
    import types
    try:
        import antenv.axon_hooks  # noqa: F401
        return
    except ImportError:
        pass
    try:
        import antenv
        mod = types.ModuleType("antenv.axon_hooks")
        _h = [None]
        mod.set_axon_ntff_profile_hook = lambda h: _h.__setitem__(0, h)
        mod.get_axon_ntff_profile_hook = lambda: _h[0]
        sys.modules["antenv.axon_hooks"] = mod
        antenv.axon_hooks = mod
        from trn_agent_boot.trn_boot import _ntff_profile_via_ctypes
        hook = _ntff_profile_via_ctypes("/opt/axon/libaxon_pjrt.so")
        if hook is not None:
            _h[0] = hook
    except Exception:
        pass


def kernel(external, prev_spikes, membrane, inter_weights, local_kernel,
           refractory, conn_src, conn_dst, _trace=False):
    if _trace:
        _ensure_ntff_hook()
    conns, R, W, groups_meta, in_maps = _prepare_inputs(
        external, prev_spikes, membrane, inter_weights, local_kernel,
        refractory, conn_src, conn_dst)
    nc = _get_program(conns, R, W, groups_meta)
    res = run_bass_kernel_spmd(nc, in_maps, core_ids=list(range(NCORES)),
                               trace=_trace)
    out = np.concatenate([r["out"].astype(np.float32) for r in res.results],
                         axis=1)
    if _trace:
        kernel._last_results = res
    return out


# revision 24
# speedup vs baseline: 2.2137x; 1.0070x over previous
"""Trainium2 Bass kernel for nn_CognitiveModule (gnn_message_passing).

Computes, for L=8 layers of a 1536x1536 grid:
  internal = conv2d(prev_spikes, local_kernel, SAME)      # 11x11 distance kernel
  axonal   = segment_sum(prev_spikes[conn_src] * inter_weights, conn_dst)
  total    = external + internal + axonal
  active   = (refractory == 0)
  v_new    = 0.9 * membrane + active * total
  spikes   = (v_new > 0) * active          (the sigmoid straight-through term
                                            cancels in the forward pass)

Strategy (8 NeuronCores, shard H):
  - Each core gets 192 rows of every layer (plus a 5-row halo of prev_spikes).
  - Conv runs on the TensorEngine as banded matmuls over the row (partition)
    dimension: for each kernel column kx, a [106,96] band matrix contracts 106
    input rows into 96 output rows.  The x-taps are reduced from 11 to 6
    matmul passes via the kernel's x-symmetry: the DVE pre-adds the shifted
    spike images (S_d = X_{-d} + X_{+d}; spikes are {0,1} so sums are exact
    in bf16).  A one-column-shifted copy (Xo) keeps every DVE op 4B-aligned
    (2x perf mode).
  - All matmul data is bf16.  Kernel/weight values are split hi/lo
    (w = bf16(w) + bf16(w - bf16(w))) so products against {0,1,2} spikes are
    exact to ~2^-18 relative - fp32-class accuracy at bf16 matmul speed.
  - external + 0.9*membrane and the refractory gate are folded on the host
    into one fp32 threshold plane  thr = BIG*(refr != 0) - (ext + 0.9*mem);
    the device finalize is ONE VectorEngine op per row-block:
        out = (psum > thr)  as {1.0, 0.0}  (bf16, cast to fp32 on the host).
  - Axonal products are computed on the VectorEngine from the resident spike
    tiles (over all 106 rows) and accumulated into PSUM with a shifted
    identity matmul (iden5: rows 5..100 -> psum rows 0..95).  Connections
    with src >= dst get a dedicated spike load at the start of each
    row-block.  The finalize is software-pipelined one layer behind the
    matmuls so the PE never waits on the DVE queue.
"""

import sys

for _p in ("/opt/trn_rl_repo", "/root/.axon_site/_ro/trn_rl_repo"):
    if _p not in sys.path:
        sys.path.append(_p)

import numpy as np
import ml_dtypes

import dataclasses

import concourse.bass as bass
import concourse.mybir as mybir
import concourse.tile as tile
from concourse import bacc
from concourse.bass_utils import run_bass_kernel_spmd

DT16 = mybir.dt.float16
NP16 = np.float16
F32 = mybir.dt.float32
BIG = np.float32(1.0e5)
DECAY = np.float32(0.9)

L = 8
NCORES = 8
TH = 96          # output rows per conv tile
HALO = 5
KS = 11          # kernel size
KR = TH + 2 * HALO  # 106 input rows per conv tile
NFREE = 512      # psum free-dim tile
WPAD = 12        # spk row padding: 5 left + 7 right (even widths, alignment)


def _split16(x):
    hi = x.astype(NP16)
    lo = (x - hi.astype(np.float32)).astype(NP16)
    return hi, lo


def _group_kernel_columns(kern):
    """Group the 11 kernel columns by x-symmetry: ("pair", d) groups read the
    pre-added S_d image; ("single", dx) groups read a shifted X window."""
    groups = []
    used = [False] * KS
    for d in range(0, HALO + 1):
        a, b = HALO + d, HALO - d
        if d == 0:
            groups.append(("pair", 0, kern[:, HALO].copy()))
            used[HALO] = True
        elif np.array_equal(kern[:, a], kern[:, b]):
            groups.append(("pair", d, kern[:, a].copy()))
            used[a] = used[b] = True
    for kx in range(KS):
        if not used[kx]:
            groups.append(("single", kx - HALO, kern[:, kx].copy()))
    return groups


def _band_matrix(col):
    """[KR, TH] band matrix: B[k, m] = col[k - m] for 0 <= k-m <= 10.
    X partition k holds spike row r0 + k - 5 (straight layout)."""
    B = np.zeros((KR, TH), np.float32)
    for m in range(TH):
        for ky in range(KS):
            B[m + ky, m] = col[ky]
    return B


def _build_program(conns, R, W, groups_meta):
    """Build the SPMD Bass program (identical on all cores)."""
    nc = bacc.Bacc(None, target_bir_lowering=False, debug=False)
    NT = W // NFREE
    HT = R // TH
    NG = len(groups_meta)

    spk_d = nc.dram_tensor("spk", [L, R + 2 * HALO, W + WPAD], DT16,
                           kind="ExternalInput")
    thr_d = nc.dram_tensor("thr", [L, R, W], F32, kind="ExternalInput")
    # single-fp16 weights, rows padded by the conv halo
    wpk_d = nc.dram_tensor("wpk", [len(conns), R + 2 * HALO, W], DT16,
                           kind="ExternalInput")
    bands_d = nc.dram_tensor("bands", [KR, 2 * NG * TH], DT16,
                             kind="ExternalInput")
    iden_d = nc.dram_tensor("iden", [KR, TH], DT16, kind="ExternalInput")
    out_d = nc.dram_tensor("out", [L, R, W], DT16, kind="ExternalOutput")

    pre_conns = [i for i, (s, d) in enumerate(conns) if s >= d]
    inline_conns = [i for i, (s, d) in enumerate(conns) if s < d]
    by_src = {}
    for i in inline_conns:
        by_src.setdefault(conns[i][0], []).append(i)
    by_dst = {}
    for i in range(len(conns)):
        by_dst.setdefault(conns[i][1], []).append(i)

    with tile.TileContext(nc) as tc:
        with (
            tc.tile_pool(name="const", bufs=1) as constp,
            tc.tile_pool(name="xp", bufs=4) as xp,
            tc.tile_pool(name="sp", bufs=2) as sp,
            tc.tile_pool(name="thrp", bufs=3) as thrp,
            tc.tile_pool(name="wp", bufs=6) as wp,
            tc.tile_pool(name="cp", bufs=12) as cp,
            tc.tile_pool(name="op", bufs=3) as op,
            tc.tile_pool(name="prep", bufs=2) as prep,
            tc.tile_pool(name="ps", bufs=2, space="PSUM") as psp,
        ):
            bands_sb = constp.tile([KR, 2 * NG * TH], DT16)
            nc.sync.dma_start(out=bands_sb[:], in_=bands_d[:])
            iden_sb = constp.tile([KR, TH], DT16)
            nc.sync.dma_start(out=iden_sb[:], in_=iden_d[:])

            # deferred finalize, one layer behind (PE never waits on DVE)
            pending = [None]

            def flush_pending():
                if pending[0] is None:
                    return
                ps_p, thr_p, out_p, store_p, l_p, r0_p = pending[0]
                nc.vector.tensor_tensor(
                    out=out_p[:], in0=ps_p[:], in1=thr_p[:],
                    op=mybir.AluOpType.is_gt)
                if store_p is not None:
                    dst = out_d[l_p - 1, r0_p:r0_p + TH, 0:W]
                    dst = dataclasses.replace(
                        dst, ap=[[W, TH], [R * W, 2], [1, W]])
                    nc.gpsimd.dma_start(
                        out=dst,
                        in_=store_p[:].rearrange("p (j x) -> p j x", x=W))
                pending[0] = None

            for h in range(HT):
                r0 = h * TH
                contrib = {}  # conn idx -> (hi_tile, lo_tile)

                # connections whose src comes later in the layer loop: load
                # the needed spike rows now (odd-aligned, all 106 rows).
                for ci in pre_conns:
                    s = conns[ci][0]
                    spre = prep.tile([KR, W + 8], DT16, tag="spre")
                    nc.gpsimd.dma_start(
                        out=spre[:],
                        in_=spk_d[s, r0:r0 + KR, 1:W + 9])
                    wt = wp.tile([KR, W], DT16, tag="w")
                    nc.gpsimd.dma_start(out=wt[:],
                                        in_=wpk_d[ci, r0:r0 + KR, :])
                    chi = cp.tile([KR, W], DT16, tag="c")
                    nc.vector.tensor_tensor(out=chi[:], in0=spre[:, 4:4 + W],
                                            in1=wt[:],
                                            op=mybir.AluOpType.mult)
                    contrib[ci] = (chi,)

                for l in range(L):
                    # paired thr load / out store: one DMA per two layers
                    if l % 2 == 0:
                        thr2 = thrp.tile([TH, 2 * W], F32, tag="thr")
                        src = thr_d[l, r0:r0 + TH, 0:W]
                        src = dataclasses.replace(
                            src, ap=[[W, TH], [R * W, 2], [1, W]])
                        nc.gpsimd.dma_start(
                            out=thr2[:].rearrange("p (j x) -> p j x", x=W),
                            in_=src)
                        out2 = op.tile([TH, 2 * W], DT16, tag="out")
                    # X[k] = spike row r0 + k - 5 (dram row r0 + k), cols at
                    # byte 2x.  Xo is shifted one column (odd alignment) for
                    # the even-d pre-adds and the contrib muls.
                    # one DMA loads both the aligned (X) and one-column-
                    # shifted (Xo) copies: src has an overlapping [1,2] dim.
                    XX = xp.tile([KR, 2 * (W + WPAD)], DT16, tag="X")
                    src = spk_d[l, r0:r0 + KR, 0:W + WPAD - 1]
                    src = dataclasses.replace(
                        src, ap=[[W + WPAD, KR], [1, 2], [1, W + WPAD - 1]])
                    dst = XX[:].rearrange("p (j x) -> p j x", x=W + WPAD)
                    nc.gpsimd.dma_start(out=dst[:, :, 0:W + WPAD - 1], in_=src)
                    X = XX[:, 0:W + WPAD]
                    Xo = XX[:, W + WPAD:2 * (W + WPAD)]

                    # symmetric pre-adds S_d = X_{-d} + X_{+d} (all DVE 2x)
                    svec = {}
                    for gi, (kind, d) in enumerate(groups_meta):
                        if kind == "pair" and d > 0:
                            S = sp.tile([KR, W], DT16, tag=f"S{d}")
                            if d % 2 == 0:
                                nc.vector.tensor_tensor(
                                    out=S[:],
                                    in0=Xo[:, 4 - d:4 - d + W],
                                    in1=Xo[:, 4 + d:4 + d + W],
                                    op=mybir.AluOpType.add)
                            else:
                                nc.vector.tensor_tensor(
                                    out=S[:], in0=X[:, HALO - d:HALO - d + W],
                                    in1=X[:, HALO + d:HALO + d + W],
                                    op=mybir.AluOpType.add)
                            svec[d] = S

                    # contrib planes for connections with src == l (dst > l)
                    # (conns are sorted, so same-src w planes are adjacent in
                    # wpk: load them with one DMA)
                    mycs = by_src.get(l, [])
                    if mycs:
                        nw = len(mycs)
                        assert mycs == list(range(mycs[0], mycs[0] + nw))
                        wt = wp.tile([KR, nw * W], DT16, tag="w")
                        src = wpk_d[mycs[0], r0:r0 + KR, 0:W]
                        src = dataclasses.replace(
                            src, ap=[[W, KR], [(R + 2 * HALO) * W, nw],
                                     [1, W]])
                        dst = wt[:].rearrange("p (j x) -> p j x", x=W)
                        nc.gpsimd.dma_start(out=dst, in_=src)
                        for k, ci in enumerate(mycs):
                            chi = cp.tile([KR, W], DT16, tag="c")
                            nc.vector.tensor_tensor(
                                out=chi[:], in0=Xo[:, 4:4 + W],
                                in1=wt[:, k * W:(k + 1) * W],
                                op=mybir.AluOpType.mult)
                            contrib[ci] = (chi,)

                    flush_pending()

                    thr_v = thr2[:, (l % 2) * W:(l % 2 + 1) * W]
                    out_v = out2[:, (l % 2) * W:(l % 2 + 1) * W]
                    store = out2 if l % 2 == 1 else None
                    my_contribs = [contrib[ci] for ci in by_dst.get(l, [])]
                    ps = psp.tile([TH, W], F32)  # 3 PSUM banks

                    for n in range(NT):
                        c0 = n * NFREE
                        n_mm = 2 * NG + sum(len(t) for t in my_contribs)
                        mm = 0
                        order = []
                        for gi, (kind, d) in enumerate(groups_meta):
                            order.append((0, gi, kind, d))
                            order.append((1, gi, kind, d))
                        for part, gi, kind, d in order:
                            lhsT = bands_sb[:, (part * NG + gi) * TH:
                                            (part * NG + gi + 1) * TH]
                            if kind == "pair" and d > 0:
                                rhs = svec[d][:, c0:c0 + NFREE]
                            else:
                                dx = 0 if kind == "pair" else d
                                rhs = X[:, HALO + dx + c0:
                                        HALO + dx + c0 + NFREE]
                            nc.tensor.matmul(ps[:, c0:c0 + NFREE], lhsT, rhs,
                                             start=(mm == 0),
                                             stop=(mm == n_mm - 1))
                            mm += 1
                        for cts in my_contribs:
                            for ct in cts:
                                nc.tensor.matmul(ps[:, c0:c0 + NFREE],
                                                 iden_sb[:],
                                                 ct[:, c0:c0 + NFREE],
                                                 start=(mm == 0),
                                                 stop=(mm == n_mm - 1))
                                mm += 1
                    pending[0] = (ps, thr_v, out_v, store, l, r0)
            flush_pending()

    nc.compile()
    return nc


_PROGRAM_CACHE = {}


def _get_program(conns, R, W, groups_meta):
    key = (tuple(conns), R, W, tuple(groups_meta))
    if key not in _PROGRAM_CACHE:
        _PROGRAM_CACHE[key] = _build_program(conns, R, W, groups_meta)
    return _PROGRAM_CACHE[key]


def _prepare_inputs(external, prev_spikes, membrane, inter_weights,
                    local_kernel, refractory, conn_src, conn_dst):
    Lx, H, W = external.shape
    R = H // NCORES
    conns = [(int(s), int(d)) for s, d in zip(conn_src, conn_dst)]
    order = sorted(range(len(conns)), key=lambda i: conns[i])
    conns = [conns[i] for i in order]

    groups = _group_kernel_columns(np.asarray(local_kernel, np.float32))
    groups_meta = [(k, d) for k, d, _c in groups]

    # band matrices, hi parts then lo parts, [KR, 2*NG*TH] bf16
    NG = len(groups)
    bands = np.zeros((KR, 2 * NG * TH), NP16)
    for gi, (_k, _d, col) in enumerate(groups):
        B = _band_matrix(col)
        hi, lo = _split16(B)
        bands[:, gi * TH:(gi + 1) * TH] = hi
        bands[:, (NG + gi) * TH:(NG + gi + 1) * TH] = lo
    # shifted identity: psum row m accumulates contrib tile row m+5
    iden = np.zeros((KR, TH), NP16)
    for m in range(TH):
        iden[m + HALO, m] = 1.0

    # fp32 threshold plane: out fires iff psum > thr
    ext = np.asarray(external, np.float32)
    mem = np.asarray(membrane, np.float32)
    refr = np.asarray(refractory)
    thr = (BIG * (refr != 0).astype(np.float32)
           - (ext + DECAY * mem)).astype(np.float32)

    # padded fp16 spikes (exact: values {0,1}); 5 left / 7 right columns
    spk = np.zeros((Lx, H + 2 * HALO, W + WPAD), NP16)
    spk[:, HALO:H + HALO, HALO:W + HALO] = np.asarray(prev_spikes, np.float32)

    # single fp16 weights (products vs {0,1} spikes are exact; the only
    # error is the ~2^-11 fp16 quantization of w itself)
    wpk = np.zeros((len(conns), H + 2 * HALO, W), NP16)
    wpk[:, HALO:H + HALO, :] = np.asarray(inter_weights,
                                          np.float32)[order]

    in_maps = []
    for c in range(NCORES):
        g0 = c * R
        in_maps.append({
            "spk": np.ascontiguousarray(spk[:, g0:g0 + R + 2 * HALO, :]),
            "thr": np.ascontiguousarray(thr[:, g0:g0 + R, :]),
            "wpk": np.ascontiguousarray(wpk[:, g0:g0 + R + 2 * HALO, :]),
            "bands": bands,
            "iden": iden,
        })
    return conns, R, W, groups_meta, in_maps


def _ensure_ntff_hook():
    """Inject the missing antenv.axon_hooks module + ctypes NTFF hook so
    trace=True works in this image (profiling only; best-effort)."""
    import types
    try:
        import antenv.axon_hooks  # noqa: F401
        return
    except ImportError:
        pass
    try:
        import antenv
        mod = types.ModuleType("antenv.axon_hooks")
        _h = [None]
        mod.set_axon_ntff_profile_hook = lambda h: _h.__setitem__(0, h)
        mod.get_axon_ntff_profile_hook = lambda: _h[0]
        sys.modules["antenv.axon_hooks"] = mod
        antenv.axon_hooks = mod
        from trn_agent_boot.trn_boot import _ntff_profile_via_ctypes
        hook = _ntff_profile_via_ctypes("/opt/axon/libaxon_pjrt.so")
        if hook is not None:
            _h[0] = hook
    except Exception:
        pass


def kernel(external, prev_spikes, membrane, inter_weights, local_kernel,
           refractory, conn_src, conn_dst, _trace=False):
    if _trace:
        _ensure_ntff_hook()
    conns, R, W, groups_meta, in_maps = _prepare_inputs(
        external, prev_spikes, membrane, inter_weights, local_kernel,
        refractory, conn_src, conn_dst)
    nc = _get_program(conns, R, W, groups_meta)
    res = run_bass_kernel_spmd(nc, in_maps, core_ids=list(range(NCORES)),
                               trace=_trace)
    out = np.concatenate([r["out"].astype(np.float32) for r in res.results],
                         axis=1)
    if _trace:
        kernel._last_results = res
    return out
